# revision 24
# baseline (speedup 1.0000x reference)
"""Trainium2 Bass kernel v3 for nn_PoissonNLLLoss (B=16, H=1024, W=2048).

Computes LOSS_WEIGHT * (mean(exp(logits)) - inst) exactly as the jax
reference. Data-parallel: 2 images per core across 8 cores; host combines
the per-core scalars.

v3 strategy (457 us vs v2's 569 us in the TimelineSim cost model):
  id' = label - 101; only id' in [0, 255) matters (ids > 100), so the
  valid-id mask reduces to cnt > 0. Radix id' = 8*hi + lo with hi in
  [0, 32), lo in [0, 8).
  v2 spent a third of its DVE/GPSIMD budget building x-scaled moving
  planes (lo-onehot * x) because tensor_tensor runs at only 2x DVE mode.
  v3 eliminates the x-planes entirely: the matmul cost model (and PE
  streaming cost) depends only on moving width, so the cnt-columns are
  streamed TWICE into position-keyed PSUM tiles:
    ps_hy[a, oct*16 + {cnt8, sy8}]  keyed by oct = x>>6   (32 keys)
    ps_rl[a, rl*8 + cnt8]           keyed by rl  = x&63   (64 keys)
  and sx is decoded per image as sum(64*oct*cnt_oct) + sum(rl*cnt_rl) -
  every term an exact integer < 2^24 in f32. PSUM start/stop is once per
  tile per image: the start bit zeroes the whole 2KB zero region, so
  interleaved per-slice starts are illegal (HW corrupts, CoreSim rejects).
  Moving planes are only {lo-onehot (4x DVE tensor_scalar sweeps),
  lo-onehot*y (ACT per-partition scale)}; stationary hi-onehot sweeps are
  split 24 on DVE / 8 on GPSIMD. Digit extraction is int16: hi via one
  biased-cast tensor_scalar, lo = id16 & 7 (invalid pixels with
  label <= 100 get hi < 0 under the HW round-to-nearest cast, so they
  never match a bin). Note the backend rejects tensor_scalar ops mixing
  bitwise op0 with arith op1, so the digit extracts cannot be folded into
  the one-hot sweeps. Engine busy/span: DVE 93%, Pool 92%, ACT 83%
  (exp + id16 + y-scales), PE 72%. Host side runs the device at least
  twice and requires bit-identical outputs (the axon-proxied device
  occasionally returns silently corrupted results).
"""

import numpy as np

P = 128
NA = 32          # hi-digit bins (stationary width)
NB = 8           # lo-digit bins
SHIFT = 101      # id' = label - SHIFT; bins cover id' in [0, 256)
NPL = 3 * NB     # moving planes: {B, B*y, B*x}

B, H, W = 16, 1024, 2048
N_CORES = 8
NIMG = B // N_CORES
G = 512          # chunk-batch width (columns per plane-tile)
NPOOL_A = 8      # hi-planes built on GPSIMD (rest on DVE)
NPL2 = 2 * NB    # moving planes actually built: {lo-onehot, lo-onehot*y}
OCTW = 64        # position-octet width: sx recovered from psum tiles keyed by
NOCT = 2048 // OCTW   # oct = x>>6 (32 tiles) ...
NRL = OCTW            # ... and rl = x&63 (64 tiles)


def _build_nc(n_img, H, W, trunc_cast=False):
    # trunc_cast: CoreSim truncates on f32->int copy; TRN2 HW rounds to
    # nearest. The hi-digit extraction bias must match the cast mode.
    hi_bias = 0.5 if trunc_cast else -(NB / 2.0 - 0.5)
    import concourse.bass as bass
    import concourse.bacc as bacc
    import concourse.tile as tile
    from concourse import mybir

    f32 = mybir.dt.float32
    f16 = mybir.dt.float16
    i32 = mybir.dt.int32
    i16 = mybir.dt.int16
    Alu = mybir.AluOpType
    Act = mybir.ActivationFunctionType

    NBAND = H // P
    NBATCH = W // G
    n_btiles = n_img * NBAND

    nc = bacc.Bacc('TRN2', target_bir_lowering=False, debug=False)
    logits_h = nc.declare_dram_parameter("logits", [n_img, H, W], f32, isOutput=False)
    label_h = nc.declare_dram_parameter("label", [n_img, H, W], i32, isOutput=False)
    out_h = nc.declare_dram_parameter("out", [1, 4], f32, isOutput=True)

    with tile.TileContext(nc) as tc:
        import contextlib
        ctx = contextlib.ExitStack()
        with ctx:
            cpool = ctx.enter_context(tc.tile_pool(name="consts", bufs=1))
            bandp = ctx.enter_context(tc.tile_pool(name="band", bufs=2))
            planep = ctx.enter_context(tc.tile_pool(name="planes", bufs=2))
            accp = ctx.enter_context(tc.tile_pool(name="acc", bufs=1))
            psum = ctx.enter_context(tc.tile_pool(name="psum", bufs=2, space="PSUM"))
            fin = ctx.enter_context(tc.tile_pool(name="fin", bufs=1))

            # ---- constants
            # per-band y scalars: y_col[p, band] = 128*band + p  (f32 ptr ok)
            y_col = cpool.tile([P, NBAND], f32)
            nc.gpsimd.iota(y_col[:], pattern=[[P, NBAND]], base=0,
                           channel_multiplier=1,
                           allow_small_or_imprecise_dtypes=True)
            ones_col = cpool.tile([P, 1], f32)
            nc.vector.memset(ones_col[:], 1.0)
            # sx decode weights: woct[(o, b)] = OCTW*o, wrl[(r, b)] = r
            woct = cpool.tile([NA, NOCT * NB], f32)
            nc.gpsimd.iota(woct[:], pattern=[[OCTW, NOCT], [0, NB]], base=0,
                           channel_multiplier=0,
                           allow_small_or_imprecise_dtypes=True)
            wrl = cpool.tile([NA, NRL * NB], f32)
            nc.gpsimd.iota(wrl[:], pattern=[[1, NRL], [0, NB]], base=0,
                           channel_multiplier=0,
                           allow_small_or_imprecise_dtypes=True)

            exp_accs = accp.tile([P, n_btiles], f32)

            # per-image accumulators in SBUF [NA, NPL]
            accs = []
            for img in range(n_img):
                a = accp.tile([NA, NPL], f32, tag=f"acc{img}")
                accs.append(a)

            bounce_h = nc.dram_tensor("bounce", [n_img * 3 * 256], f32)

            for img in range(n_img):
                # position-keyed psum accumulators:
                #   ps_hy[a, (oct, {cnt8, sy8})] keyed by oct = x>>6
                #   ps_rl[a, (rl, cnt8)]        keyed by rl = x&63
                # cnt/sy = sum over oct; sx = sum(64*oct*cnt_oct) + sum(rl*cnt_rl)
                # (all exact integer sums < 2^24 in f32)
                ps_hy = psum.tile([NA, NOCT * 2 * NB], f32, tag="ps_hy")
                ps_rl = psum.tile([NA, NRL * NB], f32, tag="ps_rl")
                for band in range(NBAND):
                    r0 = band * P
                    label_band = bandp.tile([P, W], i32, tag="label_band")
                    nc.sync.dma_start(out=label_band[:], in_=label_h[img, r0:r0 + P, :])
                    logits_band = bandp.tile([P, W], f32, tag="logits_band")
                    nc.sync.dma_start(out=logits_band[:], in_=logits_h[img, r0:r0 + P, :])

                    # exp + per-partition row-sum fused on ACT
                    exp_scr = bandp.tile([P, W], f32, tag="exp_scr")
                    nc.scalar.activation(
                        out=exp_scr[:], in_=logits_band[:], func=Act.Exp,
                        accum_out=exp_accs[:, img * NBAND + band: img * NBAND + band + 1])

                    # id' = label - 101 as i16 (exact)
                    id16 = bandp.tile([P, W], i16, tag="id16")
                    nc.scalar.activation(out=id16[:], in_=label_band[:], func=Act.Copy,
                                         bias=-float(SHIFT))
                    # hi = int_cast((id' + hi_bias) / NB)  (exact floor for id'>=0;
                    # negatives give hi < 0 under the HW round-to-nearest cast,
                    # so they never match a bin)
                    hi_i = bandp.tile([P, W], i16, tag="hi_i")
                    nc.vector.tensor_scalar(out=hi_i[:], in0=id16[:],
                                            scalar1=hi_bias, scalar2=1.0 / NB,
                                            op0=Alu.add, op1=Alu.mult)
                    # lo = id' & (NB-1)  (NB is a power of two)
                    lo = bandp.tile([P, W], i16, tag="lo")
                    nc.vector.tensor_scalar(out=lo[:], in0=id16[:],
                                            scalar1=NB - 1, scalar2=None,
                                            op0=Alu.bitwise_and)

                    for bi in range(NBATCH):
                        c0 = bi * G
                        # moving planes [P, (plane, c)]: plane = b | NB+b.
                        # lo one-hots FIRST (ACT y-scale chains on them).
                        Bt = planep.tile([P, NPL2 * G], f16, tag="Bt")
                        Bv = Bt[:].rearrange("p (q c) -> p q c", q=NPL2)
                        for b in range(NB):
                            # B_b = (lo == b)
                            nc.vector.tensor_scalar(out=Bv[:, b, :],
                                                    in0=lo[:, c0:c0 + G],
                                                    scalar1=float(b), scalar2=None,
                                                    op0=Alu.is_equal)
                        for b in range(NB):
                            # B_b * y  (per-partition scale on ACT)
                            nc.scalar.activation(out=Bv[:, NB + b, :],
                                                 in_=Bv[:, b, :], func=Act.Copy,
                                                 scale=y_col[:, band:band + 1])
                        # stationary planes: A[p, (a, c)] = (hi == a)
                        At = planep.tile([P, NA * G], f16, tag="At")
                        Av = At[:].rearrange("p (a c) -> p a c", a=NA)
                        for a in range(NA):
                            eng = nc.gpsimd if a < NPOOL_A else nc.vector
                            eng.tensor_scalar(out=Av[:, a, :], in0=hi_i[:, c0:c0 + G],
                                              scalar1=float(a), scalar2=None,
                                              op0=Alu.is_equal)
                        for g in range(G):
                            x = c0 + g
                            oct = x // OCTW
                            rl = x % OCTW
                            # psum "start" zeroes the whole 2KB zero region
                            # (= the whole tile), so exactly one start/stop per
                            # image per tile; slices in between just accumulate.
                            first = (band == 0 and x == 0)
                            last = (band == NBAND - 1 and x == W - 1)
                            nc.tensor.matmul(
                                out=ps_hy[:, oct * 2 * NB:(oct + 1) * 2 * NB],
                                lhsT=Av[:, :, g],
                                rhs=Bv[:, :, g],
                                start=first, stop=last,
                            )
                            nc.tensor.matmul(
                                out=ps_rl[:, rl * NB:(rl + 1) * NB],
                                lhsT=Av[:, :, g],
                                rhs=Bv[:, 0:NB, g],
                                start=first, stop=last,
                            )

                # ---- per-image decode of the position-keyed psums into
                # accs[img][a, (s, b)] = {cnt, sy, sx}, then bounce to DRAM so
                # the finalize reload overlaps the next image.
                Hs = planep.tile([NA, NOCT * 2 * NB], f32, tag="Hs")
                nc.vector.tensor_copy(Hs[:], ps_hy[:])
                RLs = planep.tile([NA, NRL * NB], f32, tag="RLs")
                nc.vector.tensor_copy(RLs[:], ps_rl[:])
                # cnt = sum_oct Hs[:, oct, 0:8]; sy = sum_oct Hs[:, oct, 8:16]
                Hv = Hs[:].rearrange("p (o s) -> p s o", o=NOCT)
                nc.vector.tensor_reduce(out=accs[img][:, 0:NB],
                                        in_=Hv[:, 0:NB, :],
                                        axis=mybir.AxisListType.X, op=Alu.add)
                nc.vector.tensor_reduce(out=accs[img][:, NB:2 * NB],
                                        in_=Hv[:, NB:2 * NB, :],
                                        axis=mybir.AxisListType.X, op=Alu.add)
                # sx = sum_oct 64*oct*cnt_oct + sum_rl rl*cnt_rl
                T1 = planep.tile([NA, NOCT * NB], f32, tag="T1")
                T1v = T1[:].rearrange("p (o b) -> p o b", o=NOCT)
                Hc = Hs[:].rearrange("p (o s) -> p o s", o=NOCT)
                nc.vector.tensor_tensor(out=T1v[:, :, :], in0=Hc[:, :, 0:NB],
                                        in1=woct[:].rearrange(
                                            "p (o b) -> p o b", o=NOCT),
                                        op=Alu.mult)
                T2 = planep.tile([NA, NRL * NB], f32, tag="T2")
                nc.vector.tensor_tensor(out=T2[:], in0=RLs[:], in1=wrl[:],
                                        op=Alu.mult)
                sx1 = planep.tile([NA, NB], f32, tag="sx1")
                nc.vector.tensor_reduce(out=sx1[:],
                                        in_=T1[:].rearrange(
                                            "p (o b) -> p b o", o=NOCT),
                                        axis=mybir.AxisListType.X, op=Alu.add)
                sx2 = planep.tile([NA, NB], f32, tag="sx2")
                nc.vector.tensor_reduce(out=sx2[:],
                                        in_=T2[:].rearrange(
                                            "p (r b) -> p b r", r=NRL),
                                        axis=mybir.AxisListType.X, op=Alu.add)
                nc.vector.tensor_tensor(out=accs[img][:, 2 * NB:3 * NB],
                                        in0=sx1[:], in1=sx2[:], op=Alu.add)
                nc.sync.dma_start(
                    out=bounce_h[img * 3 * 256:(img + 1) * 3 * 256]
                    .rearrange("(s a b) -> a s b", s=3, a=NA),
                    in_=accs[img][:].rearrange("a (s b) -> a s b", s=3))

            # ---- finalize on device ----
            # accs[img][a, b] = cnt, [a, NB+b] = sy, [a, 2NB+b] = sx for
            # id' = NB*a + b, id = id' + 101. Bins with id' >= 255 get cnt 0.
            # Transposed finalize layout: 256 bins -> [P=128, 2*n_img] with
            # bin = 128*half + p, column = img*2 + half.
            # Reshape accs: [NA, 3, NB] -> per (a,b): bin index q = NB*a+b.
            # We gather into [P, cols] via small DMAs through DRAM? Avoid:
            # do it with strided SBUF copies: for half in 0..1:
            #   cnt_t[p, img*2+half] = acc[(128*half + p) // NB, (128*half+p) % NB]
            # The AP for that is acc viewed as [NA*NB] with partition stride 1:
            # acc tile is [NA partitions, NPL free]; we need partition-major
            # flattening -> use 8 tiny copies per half (p = 16*a' rows...).
            # Simpler: DMA bounce through DRAM once per image (tiny).
            # reload: [P, n_img*3*2]: col = ((img*3 + s)*2 + half), row p:
            # value = stat s of bin (128*half + p) of image img
            NCOL = n_img * 3 * 2
            stats = fin.tile([P, NCOL], f32, tag="stats")
            nc.sync.dma_start(
                out=stats[:].rearrange("p (k h) -> p k h", k=n_img * 3),
                in_=bounce_h[:].rearrange("(k h p) -> p k h", k=n_img * 3, h=2))

            def col(img, s, half):
                c = (img * 3 + s) * 2 + half
                return c

            # centroids: cy = floor(sy/cnt), cx = floor(sx/cnt) via exact
            # floor division (reciprocal + correction), mask = cnt > 0.
            cnt = fin.tile([P, n_img * 2], f32, tag="cnt")
            sy = fin.tile([P, n_img * 2], f32, tag="sy")
            sx = fin.tile([P, n_img * 2], f32, tag="sx")
            for img in range(n_img):
                for half in range(2):
                    j = img * 2 + half
                    nc.vector.tensor_copy(cnt[:, j:j + 1], stats[:, col(img, 0, half):col(img, 0, half) + 1])
                    nc.vector.tensor_copy(sy[:, j:j + 1], stats[:, col(img, 1, half):col(img, 1, half) + 1])
                    nc.vector.tensor_copy(sx[:, j:j + 1], stats[:, col(img, 2, half):col(img, 2, half) + 1])

            NC2 = n_img * 2
            denom = fin.tile([P, NC2], f32, tag="denom")
            nc.vector.tensor_scalar(out=denom[:], in0=cnt[:], scalar1=1.0,
                                    scalar2=None, op0=Alu.max)
            rcp = fin.tile([P, NC2], f32, tag="rcp")
            nc.vector.reciprocal(rcp[:], denom[:])

            def floordiv(s_t, nm):
                qf = fin.tile([P, NC2], f32, tag=f"qf{nm}")
                nc.vector.tensor_tensor(out=qf[:], in0=s_t[:], in1=rcp[:], op=Alu.mult)
                qi = fin.tile([P, NC2], i32, tag=f"qi{nm}")
                nc.vector.tensor_copy(qi[:], qf[:])
                q = fin.tile([P, NC2], f32, tag=f"q{nm}")
                nc.vector.tensor_copy(q[:], qi[:])
                r = fin.tile([P, NC2], f32, tag=f"r{nm}")
                nc.vector.tensor_tensor(out=r[:], in0=q[:], in1=denom[:], op=Alu.mult)
                nc.vector.tensor_tensor(out=r[:], in0=s_t[:], in1=r[:], op=Alu.subtract)
                corr = fin.tile([P, NC2], f32, tag=f"corr{nm}")
                nc.vector.tensor_tensor(out=corr[:], in0=r[:], in1=denom[:], op=Alu.is_ge)
                nc.vector.tensor_tensor(out=q[:], in0=q[:], in1=corr[:], op=Alu.add)
                nc.vector.tensor_scalar(out=corr[:], in0=r[:], scalar1=0.0,
                                        scalar2=None, op0=Alu.is_lt)
                nc.vector.tensor_tensor(out=q[:], in0=q[:], in1=corr[:], op=Alu.subtract)
                return q

            qy = floordiv(sy, "y")
            qx = floordiv(sx, "x")

            offs_f = fin.tile([P, NC2], f32, tag="offs_f")
            nc.vector.scalar_tensor_tensor(out=offs_f[:], in0=qy[:], scalar=float(W),
                                           in1=qx[:], op0=Alu.mult, op1=Alu.add)
            mask = fin.tile([P, NC2], f32, tag="mask")
            nc.vector.tensor_scalar(out=mask[:], in0=cnt[:], scalar1=0.0,
                                    scalar2=None, op0=Alu.is_gt)
            # bins with id' >= 255 (i.e. half=1, p=127) can never have cnt>0
            # (id <= 355 -> id' <= 254), so no extra mask needed.
            nc.vector.tensor_tensor(out=offs_f[:], in0=offs_f[:], in1=mask[:],
                                    op=Alu.mult)
            offs_i = fin.tile([P, NC2], i32, tag="offs_i")
            nc.vector.tensor_copy(offs_i[:], offs_f[:])

            gath = fin.tile([P, NC2], f32, tag="gath")
            for img in range(n_img):
                for half in range(2):
                    j = img * 2 + half
                    nc.gpsimd.indirect_dma_start(
                        out=gath[:, j:j + 1],
                        out_offset=None,
                        in_=logits_h[:].rearrange("i h w -> (i h w)").unsqueeze(1),
                        in_offset=bass.IndirectOffsetOnAxis(
                            ap=offs_i[:, j:j + 1], axis=0),
                        element_offset=img * H * W,
                    )

            nc.vector.tensor_tensor(out=gath[:], in0=gath[:], in1=mask[:], op=Alu.mult)

            red = fin.tile([P, n_img + 1], f32, tag="red")
            for img in range(n_img):
                nc.vector.tensor_reduce(out=red[:, img:img + 1],
                                        in_=gath[:, img * 2:(img + 1) * 2],
                                        axis=mybir.AxisListType.X, op=Alu.add)
            nc.vector.tensor_reduce(out=red[:, n_img:n_img + 1], in_=exp_accs[:],
                                    axis=mybir.AxisListType.X, op=Alu.add)

            ps_fin = psum.tile([1, n_img + 1], f32, tag="ps_fin")
            nc.tensor.matmul(out=ps_fin[:], lhsT=ones_col[:], rhs=red[:],
                             start=True, stop=True)

            out_sb = fin.tile([1, 4], f32, tag="out_sb")
            nc.vector.memset(out_sb[:], 0.0)
            nc.vector.tensor_copy(out_sb[:, 0:1], ps_fin[:, n_img:n_img + 1])
            for img in range(n_img):
                nc.vector.tensor_copy(out_sb[:, 1 + img:2 + img], ps_fin[:, img:img + 1])
            nc.sync.dma_start(out=out_h[:], in_=out_sb[:])

    nc.compile()
    return nc


_NC_CACHE = {}


def kernel(logits, label):
    logits = np.ascontiguousarray(np.asarray(logits, dtype=np.float32))
    label = np.ascontiguousarray(np.asarray(label, dtype=np.int32))
    assert logits.shape == (B, H, W), logits.shape
    assert label.shape == (B, H, W), label.shape

    from concourse.bass_utils import run_bass_kernel_spmd

    key = (NIMG, H, W)
    if key not in _NC_CACHE:
        _NC_CACHE[key] = _build_nc(NIMG, H, W)
    nc = _NC_CACHE[key]

    in_maps = [
        {"logits": logits[c * NIMG:(c + 1) * NIMG],
         "label": label[c * NIMG:(c + 1) * NIMG]}
        for c in range(N_CORES)
    ]
    import time as _time

    def run_once():
        # the axon-proxied device occasionally reports a transient
        # NRT_EXEC_UNIT_UNRECOVERABLE; retry a few times before giving up
        last_exc = None
        for attempt in range(4):
            try:
                res = run_bass_kernel_spmd(nc, in_maps, list(range(N_CORES)))
                return [np.array(res.results[c]["out"][0], dtype=np.float32).copy()
                        for c in range(N_CORES)]
            except Exception as e:
                last_exc = e
                _time.sleep(2.0 * (attempt + 1))
        raise last_exc

    # the device also very occasionally returns silently-corrupted results;
    # require two bit-identical runs before trusting the output
    outs = [run_once()]
    for _ in range(4):
        outs.append(run_once())
        a, b = outs[-2], outs[-1]
        if all(np.array_equal(x, y) for x, y in zip(a, b)):
            break
        match = None
        for prev in outs[:-1]:
            if all(np.array_equal(x, y) for x, y in zip(prev, outs[-1])):
                match = prev
                break
        if match is not None:
            break
    good = outs[-1]

    exp_total = 0.0
    inst_total = 0.0
    for c in range(N_CORES):
        o = good[c]
        exp_total += float(o[0])
        for i in range(NIMG):
            inst_total += float(o[1 + i])
    int_loss = exp_total / float(B * H * W)
    inst = inst_total / float(B)
    return np.float32(int_loss - inst)

